# revision 1
# baseline (speedup 1.0000x reference)
"""Trainium2 Bass kernel for a 2-layer LSTM encoder returning final (h, c).

Problem: enc_inp [B=128, T=1024, F=64]; two stacked LSTM layers with H=128.
Layer 1 starts from zero state; layer 2's initial state is layer 1's final
state, so the 2048 recurrence steps are strictly sequential.

Strategy (per NeuronCore, 8 cores data-parallel over batch, 16 samples each):
 - Layout: hidden/gate dim on partitions, batch on the free dim. Per-step
   gate pre-activations live in PSUM as [128, 4 gates * 16 batch].
 - Input contributions (W @ x_t for all t of a 32-step chunk) are computed by
   wide GEMMs (N=512) into PSUM ahead of the recurrence; the per-step U @ h
   matmuls accumulate on top (4 matmuls/step, bf16).
 - All four gates go through ONE sigmoid ACT per step; tanh(g) is obtained
   via tanh(x) = 2*sigmoid(2x) - 1 with the g-gate weights pre-scaled by 2 on
   the host, so the gate columns are reordered to (i, f, o, g2).
 - Cell update uses fused scalar_tensor_tensor DVE ops; tanh(c) is the only
   other ACT per step. h is produced in bf16 (feeds the next matmul), c stays
   fp32.
 - Layer-0 biases ride an appended ones-row in the input (K=65); layer-1
   biases (when nonzero) use a rank-1 accumulate matmul.
"""

import numpy as np
import ml_dtypes

import concourse.bacc as bacc
import concourse.tile as tile
import concourse.mybir as mybir
from concourse.bass_utils import run_bass_kernel_spmd

N_CORES = 8
B, T_FULL, F, H = 128, 1024, 64, 128
BS = B // N_CORES  # batch per core
G4 = 4 * H
CHUNK = 32  # recurrence steps per PSUM chunk (one bank per gate)

BF16 = ml_dtypes.bfloat16

# Column permutation: keras gate order (i, f, g, o) -> (i, f, o, g)
_PERM = np.concatenate(
    [np.arange(0, H), np.arange(H, 2 * H), np.arange(3 * H, 4 * H),
     np.arange(2 * H, 3 * H)]
)

_ALU = mybir.AluOpType
_ACT = mybir.ActivationFunctionType


def _build(T, has_b1):
    """Build the SPMD Bass program for a T-step 2-layer LSTM."""
    bf = mybir.dt.bfloat16
    f32 = mybir.dt.float32

    nc = bacc.Bacc("TRN2", target_bir_lowering=False, debug=False,
                   enable_asserts=True, num_devices=N_CORES)

    xT = nc.dram_tensor("xT", [F + 1, T * BS], bf, kind="ExternalInput").ap()
    w0 = nc.dram_tensor("w0", [F + 1, G4], bf, kind="ExternalInput").ap()
    u0 = nc.dram_tensor("u0", [H, G4], bf, kind="ExternalInput").ap()
    w1 = nc.dram_tensor("w1", [H, G4], bf, kind="ExternalInput").ap()
    u1 = nc.dram_tensor("u1", [H, G4], bf, kind="ExternalInput").ap()
    if has_b1:
        b1 = nc.dram_tensor("b1", [1, G4], bf, kind="ExternalInput").ap()
    hc = nc.dram_tensor("hc", [H, 2 * BS], f32, kind="ExternalOutput").ap()

    n_chunks = (T + CHUNK - 1) // CHUNK
    assert T % CHUNK == 0

    with tile.TileContext(nc) as tc:
        with (
            tc.tile_pool(name="big", bufs=1) as big,
            tc.tile_pool(name="wts", bufs=1) as wts,
            tc.tile_pool(name="state", bufs=1) as state,
            tc.tile_pool(name="gates", bufs=4) as gates,
            tc.tile_pool(name="tmps", bufs=4) as tmps,
            tc.tile_pool(name="hsmall", bufs=4) as hsmall,
            tc.tile_pool(name="pz", bufs=2, space="PSUM") as pzpool,
        ):
            # --- load inputs ---
            xTs = big.tile([F + 1, T * BS], bf, tag="xT")
            nc.sync.dma_start(out=xTs, in_=xT)
            hs0 = big.tile([H, T * BS], bf, tag="hs0")

            w0s = wts.tile([F + 1, G4], bf, tag="w0")
            u0s = wts.tile([H, G4], bf, tag="u0")
            w1s = wts.tile([H, G4], bf, tag="w1")
            u1s = wts.tile([H, G4], bf, tag="u1")
            nc.sync.dma_start(out=w0s, in_=w0)
            nc.sync.dma_start(out=u0s, in_=u0)
            nc.sync.dma_start(out=w1s, in_=w1)
            nc.sync.dma_start(out=u1s, in_=u1)
            b1s = None
            ones = None
            if has_b1:
                b1s = wts.tile([1, G4], bf, tag="b1")
                nc.sync.dma_start(out=b1s, in_=b1)
                ones = state.tile([1, BS], bf, tag="ones")
                nc.vector.memset(ones, 1.0)

            c = state.tile([H, BS], f32, tag="c")
            nc.vector.memset(c, 0.0)
            h0 = state.tile([H, BS], bf, tag="h0")
            nc.vector.memset(h0, 0.0)
            hc_stage = state.tile([H, 2 * BS], f32, tag="hc_stage")

            def emit_chunk_gemms(pz, w_s, x_s, c0):
                """xz GEMMs for steps [c0, c0+CHUNK) of a layer into psum."""
                pz3 = pz.rearrange("p (g n) -> p g n", g=4)
                cols = slice(c0 * BS, (c0 + CHUNK) * BS)
                for j in range(4):
                    nc.tensor.matmul(
                        pz3[:, j, :],
                        w_s[:, j * H:(j + 1) * H],
                        x_s[:, cols],
                        start=True, stop=False, skip_group_check=True,
                    )

            def emit_layer(layer, x_s, w_s, u_s, b_s, h_prev, last_layer):
                pz_cur = pzpool.tile([H, 4 * CHUNK * BS], f32, tag="pz")
                emit_chunk_gemms(pz_cur, w_s, x_s, 0)
                pz_next = None
                for t in range(T):
                    k = t % CHUNK
                    if k == 0 and t > 0:
                        pz_cur = pz_next
                    pz3 = pz_cur.rearrange("p (g n) -> p g n", g=4)
                    sl = slice(k * BS, (k + 1) * BS)
                    for j in range(4):
                        nc.tensor.matmul(
                            pz3[:, j, sl],
                            u_s[:, j * H:(j + 1) * H],
                            h_prev,
                            start=False, stop=not (b_s is not None),
                            skip_group_check=True,
                        )
                        if b_s is not None:
                            nc.tensor.matmul(
                                pz3[:, j, sl],
                                b_s[:, j * H:(j + 1) * H],
                                ones,
                                start=False, stop=True, skip_group_check=True,
                            )
                    # prefetch next chunk's input GEMMs mid-chunk
                    if k == CHUNK // 2 and t + CHUNK <= T - 1:
                        pz_next = pzpool.tile([H, 4 * CHUNK * BS], f32, tag="pz")
                        emit_chunk_gemms(pz_next, w_s, x_s,
                                         (t // CHUNK + 1) * CHUNK)

                    S = gates.tile([H, 4 * BS], f32, tag="S")
                    S3 = S.rearrange("p (g n) -> p g n", g=4)
                    nc.scalar.activation(S3, pz3[:, :, sl], _ACT.Sigmoid)
                    # S columns: [sig(i) | sig(f) | sig(o) | sig(2 zg)]
                    si = S[:, 0:BS]
                    sf = S[:, BS:2 * BS]
                    so = S[:, 2 * BS:3 * BS]
                    sg = S[:, 3 * BS:4 * BS]
                    ig2 = tmps.tile([H, BS], f32, tag="ig2")
                    # ig2 = (sig(2zg) - 0.5) * i  ==  i * tanh(zg) / 2
                    nc.vector.scalar_tensor_tensor(
                        ig2, sg, 0.5, si, _ALU.subtract, _ALU.mult)
                    fc = tmps.tile([H, BS], f32, tag="fc")
                    nc.vector.tensor_mul(fc, c, sf)
                    # c = 2*ig2 + fc
                    nc.vector.scalar_tensor_tensor(
                        c, ig2, 2.0, fc, _ALU.mult, _ALU.add)
                    th = tmps.tile([H, BS], f32, tag="th")
                    nc.scalar.activation(th, c, _ACT.Tanh)
                    last_step = last_layer and t == T - 1
                    if last_step:
                        nc.vector.tensor_mul(hc_stage[:, 0:BS], th, so)
                    elif layer == 0:
                        h_prev = hs0[:, t * BS:(t + 1) * BS]
                        nc.vector.tensor_mul(h_prev, th, so)
                    else:
                        h_prev = hsmall.tile([H, BS], bf, tag="h1")
                        nc.vector.tensor_mul(h_prev, th, so)
                return h_prev

            hlast0 = emit_layer(0, xTs, w0s, u0s, None, h0, last_layer=False)
            hlast = emit_layer(1, hs0, w1s, u1s, b1s, hlast0, last_layer=True)
            del hlast
            nc.vector.tensor_copy(hc_stage[:, BS:2 * BS], c)
            nc.sync.dma_start(out=hc, in_=hc_stage)

    nc.finalize()
    return nc


_CACHE = {}


def _get_program(T, has_b1):
    key = (T, has_b1)
    if key not in _CACHE:
        _CACHE[key] = _build(T, has_b1)
    return _CACHE[key]


def _prep_weights(W0, U0, b0, W1, U1, b1):
    """Permute gates to (i,f,o,g), scale g-block by 2, cast bf16."""
    def prep(M):
        Mp = np.asarray(M, np.float32)[..., _PERM].copy()
        Mp[..., 3 * H:4 * H] *= 2.0
        return Mp
    w0a = np.concatenate([prep(W0), prep(b0)[None, :]], axis=0).astype(BF16)
    u0a = prep(U0).astype(BF16)
    w1a = prep(W1).astype(BF16)
    u1a = prep(U1).astype(BF16)
    b1p = prep(b1)[None, :].astype(BF16)
    has_b1 = bool(np.any(np.asarray(b1) != 0))
    return w0a, u0a, w1a, u1a, b1p, has_b1


def _prep_x(enc_inp, T):
    """Per-core transposed+augmented inputs: [F+1, T*BS] bf16."""
    outs = []
    for k in range(N_CORES):
        xk = np.asarray(enc_inp[k * BS:(k + 1) * BS, :T], np.float32)
        xk = np.ascontiguousarray(xk.transpose(2, 1, 0)).reshape(F, T * BS)
        xa = np.concatenate([xk, np.ones((1, T * BS), np.float32)], axis=0)
        outs.append(xa.astype(BF16))
    return outs


def run_lstm(enc_inp, W0, U0, b0, W1, U1, b1, T=T_FULL):
    w0a, u0a, w1a, u1a, b1p, has_b1 = _prep_weights(W0, U0, b0, W1, U1, b1)
    xs = _prep_x(enc_inp, T)
    nc = _get_program(T, has_b1)
    in_maps = []
    for k in range(N_CORES):
        m = {"xT": xs[k], "w0": w0a, "u0": u0a, "w1": w1a, "u1": u1a}
        if has_b1:
            m["b1"] = b1p
        in_maps.append(m)
    res = run_bass_kernel_spmd(nc, in_maps, list(range(N_CORES)))
    h = np.empty((B, H), np.float32)
    c = np.empty((B, H), np.float32)
    for k in range(N_CORES):
        hck = res.results[k]["hc"]  # [H, 2*BS]
        h[k * BS:(k + 1) * BS] = hck[:, :BS].T
        c[k * BS:(k + 1) * BS] = hck[:, BS:].T
    return h, c


def kernel(enc_inp, W0, U0, b0, W1, U1, b1):
    h, c = run_lstm(np.asarray(enc_inp), np.asarray(W0), np.asarray(U0),
                    np.asarray(b0), np.asarray(W1), np.asarray(U1),
                    np.asarray(b1), T=T_FULL)
    return h, c
